# revision 1
# baseline (speedup 1.0000x reference)
"""Trainium2 Bass kernel for the AttentionIAM block (GroupNorm + 8-head
self-attention + residual projection) on [8, 512, 32, 32] inputs.

Sharding: pure data-parallel — one batch sample per NeuronCore (8 cores).

Per-core math (C=512, T=1024, heads=8, ch=64), all on one core:
  normed = GroupNorm32(x) * gn_w + gn_b          (stats via mask matmuls)
  q = Wq' @ normed + bq'   (Wq' pre-scaled by 1/sqrt(ch) on host)
  k = Wk @ normed + bk
  vT = normed^T @ Wv^T                            (v emitted transposed)
  per head pair, per st: wT[s,t] for both heads -> one [128,1024] PSUM
     group -> single Exp ACT op (FD=1024) -> f32r expw
     a_h[c,t] + den_h via merged [v|ones] stationary accumulation
     normalize straight out of PSUM: rec = 1/den, a_h = a-hat_h * rec
  out = pwT.T @ (x + a) + (proj_b + proj_w @ bv)  (v-bias folded via softmax sum=1)

All big matmuls run in float32r. Head pairs use PE row-tiling (base
partition 0/64) so the two K=64 QK^T matmuls of a pair can overlap in the
array. exp is batched to FD=1024 (two PSUM banks per ACT op) to amortize
the ~220-cycle ACT op overhead. q/k/proj PSUM evictions ride on DVE
(tensor_scalar_add) to keep ACT free for the exp stream.
"""

import sys
import numpy as np

sys.path.insert(0, "/opt/trn_rl_repo")

B, C, T = 8, 512, 1024
H, W = 32, 32
NH, CH = 8, 64  # heads, channels/head
NG, GS = 32, 16  # groups, channels/group
EPS = 1e-5
P = 128
CT = C // P  # 4 channel tiles
TT = T // P  # 8 t tiles
NCHUNK = T // 512  # 2 free-dim chunks

_CACHE = {}
USE_FP8_ACC = True


def round_fp32r(a: np.ndarray) -> np.ndarray:
    """Round-to-nearest-even to 11 mantissa bits (the f32r format)."""
    u = np.ascontiguousarray(a, dtype=np.float32).view(np.uint32).astype(np.uint64)
    r = (u + 0x7FF + ((u >> 12) & 1)) & 0xFFFFF000
    return r.astype(np.uint32).view(np.float32)


def _build(loop_n=None, ablate=None):
    import concourse.bacc as bacc
    import concourse.tile as tile
    from concourse import mybir

    F32 = mybir.dt.float32
    F32R = mybir.dt.float32r
    F8 = mybir.dt.float8e4
    EWDT = F8 if USE_FP8_ACC else F32R
    AF = mybir.ActivationFunctionType
    OP = mybir.AluOpType
    PM_DR = mybir.MatmulPerfMode.DoubleRow

    nc = bacc.Bacc("TRN2", target_bir_lowering=False, debug=False)

    xin = nc.dram_tensor("xin", [C, T], F32, kind="ExternalInput").ap()
    wqkvT = nc.dram_tensor("wqkvT", [C, 3 * C], F32R, kind="ExternalInput").ap()
    pwT = nc.dram_tensor("pwT", [C, C], F32R, kind="ExternalInput").ap()
    # per-channel vectors: [ct, 128, 5] = (bq, bk, gn_w, gn_b, proj_b')
    vecs = nc.dram_tensor("vecs", [CT, P, 5], F32, kind="ExternalInput").ap()
    maskD = nc.dram_tensor("maskD", [C, NG], F32, kind="ExternalInput").ap()
    maskU = nc.dram_tensor("maskU", [NG, C], F32, kind="ExternalInput").ap()
    out_d = nc.dram_tensor("out", [C, T], F32, kind="ExternalOutput").ap()

    with tile.TileContext(nc) as tc:
        with (
            tc.tile_pool(name="const", bufs=1) as constp,
            tc.tile_pool(name="xp", bufs=1) as xp,
            tc.tile_pool(name="wp", bufs=1) as wp,
            tc.tile_pool(name="np_", bufs=1) as npool,
            tc.tile_pool(name="qkp", bufs=1) as qkp,
            tc.tile_pool(name="vtp", bufs=1) as vtp,
            tc.tile_pool(name="ap_", bufs=1) as apool,
            tc.tile_pool(name="small", bufs=2) as small,
            tc.tile_pool(name="expp", bufs=4) as expp,
            tc.tile_pool(name="recp", bufs=3) as recp,
            # PSUM: psw 2x[P,1024] (4 banks) + acc 3x[P,512] + fill 1x[P,512]
            tc.tile_pool(name="psw", bufs=2, space="PSUM") as pswp,
            tc.tile_pool(name="psacc", bufs=3, space="PSUM") as psacc,
            tc.tile_pool(name="psfill", bufs=1, space="PSUM") as psfill,
        ):
            # ---- loop-invariant loads (weights/biases/masks), emitted once ----
            w_sb = []
            pw_sb = []
            vec_sb = []
            mD_sb = []
            weng = [nc.sync, nc.sync, nc.scalar, nc.scalar]
            for i in range(CT):
                wt = wp.tile([P, 3 * C], F32R, name=f"w{i}")
                weng[i].dma_start(out=wt, in_=wqkvT[i * P : (i + 1) * P, :])
                w_sb.append(wt)
            for i in range(CT):
                pt = wp.tile([P, C], F32R, name=f"pw{i}")
                nc.gpsimd.dma_start(out=pt, in_=pwT[i * P : (i + 1) * P, :])
                pw_sb.append(pt)
            for i in range(CT):
                vt_ = constp.tile([P, 5], F32, name=f"vec{i}")
                nc.scalar.dma_start(out=vt_, in_=vecs[i])
                vec_sb.append(vt_)
                md = constp.tile([P, NG], F32, name=f"mD{i}")
                nc.scalar.dma_start(out=md, in_=maskD[i * P : (i + 1) * P, :])
                mD_sb.append(md)
            mU_sb = constp.tile([NG, C], F32, name="mU")
            nc.scalar.dma_start(out=mU_sb, in_=maskU)
            eps_sb = constp.tile([NG, 1], F32, name="eps")
            nc.vector.memset(eps_sb, EPS)

            def body():
                # ---- per-iteration input load ----
                x_sb = []
                xeng = [nc.gpsimd, nc.gpsimd, nc.sync, nc.sync]
                for i in range(CT):
                    xt = xp.tile([P, T], F32, name=f"x{i}", tag=f"x{i}", bufs=2)
                    xeng[i].dma_start(out=xt, in_=xin[i * P : (i + 1) * P, :])
                    x_sb.append(xt)

                if ablate == "dmaonly":
                    for i in range(CT):
                        ot = qkp.tile([P, T], F32, name=f"o{i}")
                        nc.vector.tensor_copy(out=ot, in_=x_sb[i])
                        nc.sync.dma_start(out=out_d[i * P : (i + 1) * P, :], in_=ot)
                    return

                # ---- GroupNorm stats ----
                # per-channel (mean, E[x^2]) -> mask-matmul group reduce -> [32, 2]
                psg_t = psfill.tile([P, 512], F32, name="psg", tag="fill")
                psg = psg_t[0:NG, 0:2]
                for i in range(CT):
                    bns = small.tile([P, 2, 6], F32, name="bns", tag="bns")
                    nc.vector.bn_stats(out=bns[:, 0, :], in_=x_sb[i][:, 0:512])
                    nc.vector.bn_stats(out=bns[:, 1, :], in_=x_sb[i][:, 512:1024])
                    mv = small.tile([P, 2], F32, name="mv", tag="mv")
                    nc.vector.bn_aggr(out=mv, in_=bns)
                    st_ = small.tile([P, 2], F32, name="st", tag="st")
                    nc.vector.tensor_copy(out=st_[:, 0:1], in_=mv[:, 0:1])
                    # E[x^2] = var + mean^2
                    nc.vector.tensor_mul(out=st_[:, 1:2], in0=mv[:, 0:1], in1=mv[:, 0:1])
                    nc.vector.tensor_add(out=st_[:, 1:2], in0=st_[:, 1:2], in1=mv[:, 1:2])
                    nc.tensor.matmul(psg, lhsT=mD_sb[i], rhs=st_, start=(i == 0), stop=(i == CT - 1))
                # group stats -> (mean_g, rstd_g) in SBUF [32, 2]
                gsb = small.tile([NG, 2], F32, name="gsb", tag="gsb", bufs=1)
                nc.vector.tensor_copy(out=gsb, in_=psg)
                gs = small.tile([NG, 2], F32, name="gs", tag="gs", bufs=1)
                nc.vector.tensor_copy(out=gs[:, 0:1], in_=gsb[:, 0:1])
                gvar = small.tile([NG, 1], F32, name="gvar", tag="gvar", bufs=1)
                glog = small.tile([NG, 1], F32, name="glog", tag="glog", bufs=1)
                nc.vector.tensor_mul(out=gvar, in0=gsb[:, 0:1], in1=gsb[:, 0:1])
                nc.vector.tensor_sub(out=gvar, in0=gsb[:, 1:2], in1=gvar)
                # rstd = exp(-0.5*ln(var+eps)): Ln+Exp live in one ACT table
                # set, so no per-iteration table switches (Sqrt would force 2)
                nc.scalar.activation(out=glog, in_=gvar, func=AF.Ln, bias=eps_sb, scale=1.0)
                nc.scalar.activation(out=gs[:, 1:2], in_=glog, func=AF.Exp, bias=0.0, scale=-0.5)

                # broadcast to channels + affine coefficients; normed = x*A + B
                n_sb = []
                for i in range(CT):
                    psb_t = psfill.tile([P, 512], F32, name="psb", tag="fill")
                    psb = psb_t[:, 0:2]
                    nc.tensor.matmul(psb, lhsT=mU_sb[:, i * P : (i + 1) * P], rhs=gs, start=True, stop=True)
                    coefA = small.tile([P, 1], F32, name="coefA", tag="coefA")
                    coefB = small.tile([P, 1], F32, name="coefB", tag="coefB")
                    # A = rstd_c * gn_w ; B = gn_b - mean_c * A
                    nc.vector.tensor_mul(out=coefA, in0=psb[:, 1:2], in1=vec_sb[i][:, 2:3])
                    nc.vector.tensor_mul(out=coefB, in0=psb[:, 0:1], in1=coefA)
                    nc.vector.tensor_sub(out=coefB, in0=vec_sb[i][:, 3:4], in1=coefB)
                    nt = npool.tile([P, T], F32R, name=f"normed{i}", tag=f"normed{i}", bufs=2)
                    nc.vector.tensor_scalar(
                        out=nt, in0=x_sb[i], scalar1=coefA, scalar2=coefB,
                        op0=OP.mult, op1=OP.add,
                    )
                    n_sb.append(nt)

                if ablate == "gnonly":
                    for i in range(CT):
                        ot = qkp.tile([P, T], F32, name=f"o{i}")
                        nc.vector.tensor_copy(out=ot, in_=n_sb[i])
                        nc.sync.dma_start(out=out_d[i * P : (i + 1) * P, :], in_=ot)
                    return

                # ---- qkv ----
                q_sb = [qkp.tile([P, T], F32R, name=f"q{i}") for i in range(CT)]
                k_sb = [qkp.tile([P, T], F32R, name=f"k{i}") for i in range(CT)]
                a_sb = [apool.tile([P, T], F32, name=f"a{i}") for i in range(CT)]
                r_sb = [None] * CT

                def qk_group_wide(oc):
                    # both t-chunks in one [P, 1024] PSUM tile; ci outer /
                    # tch inner so each stationary w-slice serves 2 matmuls.
                    # One DVE evict (bias add).
                    dest = q_sb[oc] if oc < CT else k_sb[oc - CT]
                    bias = vec_sb[oc % CT][:, 0:1] if oc < CT else vec_sb[oc % CT][:, 1:2]
                    ps = pswp.tile([P, 1024], F32, name="psqk", tag="psw")
                    for ci in range(CT):
                        for tch in range(NCHUNK):
                            nc.tensor.matmul(
                                ps[:, tch * 512 : (tch + 1) * 512],
                                lhsT=w_sb[ci][:, oc * P : (oc + 1) * P],
                                rhs=n_sb[ci][:, tch * 512 : (tch + 1) * 512],
                                start=(ci == 0), stop=(ci == CT - 1),
                            )
                    # ACT is idle pre-attention; Identity+bias evict is free
                    nc.scalar.activation(
                        out=dest, in_=ps, func=AF.Identity, bias=bias, scale=1.0,
                    )

                def qk_group_fill(oc, tch):
                    # one t-chunk in the single-bank fill tile (used as
                    # attention-phase filler so psw stays free for scores)
                    dest = q_sb[oc] if oc < CT else k_sb[oc - CT]
                    bias = vec_sb[oc % CT][:, 0:1] if oc < CT else vec_sb[oc % CT][:, 1:2]
                    ps = psfill.tile([P, 512], F32, name="psqkf", tag="fill")
                    for ci in range(CT):
                        nc.tensor.matmul(
                            ps,
                            lhsT=w_sb[ci][:, oc * P : (oc + 1) * P],
                            rhs=n_sb[ci][:, tch * 512 : (tch + 1) * 512],
                            start=(ci == 0), stop=(ci == CT - 1),
                        )
                    nc.vector.tensor_scalar_add(
                        out=dest[:, tch * 512 : (tch + 1) * 512], in0=ps, scalar1=bias
                    )

                # q0/k0 before attention (2-bank psw tiles, no filler pressure)
                qk_group_wide(0)
                qk_group_wide(CT)

                # vT tiles: [v_even | ones | v_odd] blocks of 192 cols per head
                # pair; merged stationary computes a-hat AND the softmax
                # denominator (pre-broadcast to 64 rows) in one matmul.
                # fp8 mode packs st-pairs [P, 2, 768] for DoubleRow matmuls.
                if USE_FP8_ACC:
                    vt2_sb = [
                        vtp.tile([P, 2, 4 * 192], F8, name=f"vt2_{jp}")
                        for jp in range(TT // 2)
                    ]
                    for jp in range(TT // 2):
                        vtv2 = vt2_sb[jp].rearrange("p s (b e) -> p s b e", e=192)
                        nc.vector.memset(vtv2[:, :, :, 64:128], 1.0)

                    def vt_build(j):
                        vtv = vt2_sb[j // 2].rearrange("p s (b e) -> p s b e", e=192)[:, j % 2]
                        ps = psacc.tile([P, 512], F32, name="psvt", tag="acc")
                        for ci in range(CT):
                            nc.tensor.matmul(
                                ps,
                                lhsT=n_sb[ci][:, j * P : (j + 1) * P],
                                rhs=w_sb[ci][:, 2 * C : 3 * C],
                                start=(ci == 0), stop=(ci == CT - 1),
                            )
                        psv = ps.rearrange("p (h e) -> p h e", e=CH)
                        nc.vector.tensor_copy(out=vtv[:, :, 0:64], in_=psv[:, 0::2, :])
                        nc.vector.tensor_copy(out=vtv[:, :, 128:192], in_=psv[:, 1::2, :])

                    # jp 0-1 up front; jp 2-3 woven into pair 0's attention
                    for j in range(4):
                        vt_build(j)
                    vt_fillers = [lambda j=j: vt_build(j) for j in range(4, TT)]
                else:
                    vt_fillers = []
                    ones_sb = constp.tile([P, 4, CH], F32, name="onesb")
                    nc.vector.memset(ones_sb, 1.0)
                    vt_sb = [vtp.tile([P, 4 * 192], F32R, name=f"vt{j}") for j in range(TT)]
                    for j in range(TT):
                        vtv = vt_sb[j].rearrange("p (b e) -> p b e", e=192)
                        nc.vector.tensor_copy(out=vtv[:, :, 64:128], in_=ones_sb)
                        ps = psacc.tile([P, 512], F32, name="psvt", tag="acc")
                        for ci in range(CT):
                            nc.tensor.matmul(
                                ps,
                                lhsT=n_sb[ci][:, j * P : (j + 1) * P],
                                rhs=w_sb[ci][:, 2 * C : 3 * C],
                                start=(ci == 0), stop=(ci == CT - 1),
                            )
                        psv = ps.rearrange("p (h e) -> p h e", e=CH)
                        nc.scalar.copy(out=vtv[:, :, 0:64], in_=psv[:, 0::2, :])
                        nc.scalar.copy(out=vtv[:, :, 128:192], in_=psv[:, 1::2, :])

                def attention_pair(hp, fillers=None):
                    # acc2[0] = [a-hat_A (0:64); den_A (64:128)]
                    # acc2[1] = [den_B (0:64); a-hat_B (64:128)]
                    for tch in range(NCHUNK):
                        tsl = slice(tch * 512, (tch + 1) * 512)
                        acc2 = [
                            psacc.tile([P, 512], F32, name=f"acc{h}", tag="acc")
                            for h in range(2)
                        ]

                        def emit_front(st):
                            # both heads' scores into one 2-bank PSUM tile;
                            # the K=64 matmuls auto-tile to PE row groups 0/64
                            pw_ = pswp.tile([P, 1024], F32, name="psw", tag="psw")
                            for h in range(2):
                                hb = h * CH
                                nc.tensor.matmul(
                                    pw_[:, h * 512 : (h + 1) * 512],
                                    lhsT=k_sb[hp][hb : hb + CH, st * P : (st + 1) * P],
                                    rhs=q_sb[hp][hb : hb + CH, tsl],
                                    start=True, stop=True,
                                )
                            ew = expp.tile([P, 1024], F32R, name="expw", tag="expw", bufs=3)
                            nc.scalar.activation(
                                out=ew, in_=pw_, func=AF.Exp, bias=0.0, scale=1.0,
                            )
                            return ew

                        def emit_acc(st, ew):
                            first, last = st == 0, st == TT - 1
                            for h in range(2):
                                # lhsT: head even -> [v|ones] cols 0:128 of its
                                # 192 block; head odd -> [ones|v] cols 64:192.
                                b0 = hp * 192 + h * CH
                                nc.tensor.matmul(
                                    acc2[h],
                                    lhsT=vt_sb[st][:, b0 : b0 + P],
                                    rhs=ew[:, h * 512 : (h + 1) * 512],
                                    start=first, stop=last,
                                )

                        def emit_front8(h, jp):
                            # one head, two adjacent s-tiles -> the exp output
                            # tile is directly the [P, 2, 512] DoubleRow rhs
                            hb = h * CH
                            pw_ = pswp.tile([P, 1024], F32, name="psw", tag="psw")
                            for sp in range(2):
                                st = 2 * jp + sp
                                nc.tensor.matmul(
                                    pw_[:, sp * 512 : (sp + 1) * 512],
                                    lhsT=k_sb[hp][hb : hb + CH, st * P : (st + 1) * P],
                                    rhs=q_sb[hp][hb : hb + CH, tsl],
                                    start=True, stop=True,
                                )
                            ew = expp.tile([P, 1024], F8, name="expw", tag="expw")
                            nc.scalar.activation(
                                out=ew, in_=pw_, func=AF.Exp, bias=0.0, scale=1.0,
                            )
                            return ew

                        def emit_acc8(h, jp, ew):
                            b0 = hp * 192 + h * CH
                            nc.tensor.matmul(
                                acc2[h],
                                lhsT=vt2_sb[jp][:, :, b0 : b0 + P],
                                rhs=ew.rearrange("p (s t) -> p s t", t=512),
                                start=(jp == 0), stop=(jp == TT // 2 - 1),
                                perf_mode=PM_DR,
                            )

                        if ablate == "noattn":
                            nc.vector.memset(acc2[0], 0.5)
                            nc.vector.memset(acc2[1], 0.5)
                        elif USE_FP8_ACC:
                            pend = []
                            for jp in range(TT // 2):
                                for h in range(2):
                                    ew = emit_front8(h, jp)
                                    pend.append((h, jp, ew))
                                    if len(pend) > 2:
                                        emit_acc8(*pend.pop(0))
                                    if fillers and jp >= 1:
                                        fillers.pop(0)()
                            for p_ in pend:
                                emit_acc8(*p_)
                        else:
                            pend = []
                            for st in range(TT):
                                ew = emit_front(st)
                                pend.append((st, ew))
                                if len(pend) > 1:
                                    emit_acc(*pend.pop(0))
                            for p_ in pend:
                                emit_acc(*p_)

                        # normalize straight out of PSUM:
                        # rec = 1/den (custom DVE op needs all 128 partitions)
                        rec = recp.tile([P, 512], F32, name="rec", tag="rec")
                        nc.vector.tensor_copy(out=rec[0:CH, :], in_=acc2[0][CH:P, :])
                        nc.vector.tensor_copy(out=rec[CH:P, :], in_=acc2[1][0:CH, :])
                        nc.vector.reciprocal_approx_fast(out=rec, in_=rec)
                        nc.vector.tensor_mul(
                            out=a_sb[hp][0:CH, tsl], in0=acc2[0][0:CH, :], in1=rec[0:CH, :]
                        )
                        nc.vector.tensor_mul(
                            out=a_sb[hp][CH:P, tsl], in0=acc2[1][CH:P, :], in1=rec[CH:P, :]
                        )

                if ablate != "qkvonly":
                    for hp in range(NH // 2):
                        attention_pair(hp, fillers=vt_fillers if hp == 0 else None)
                        # next pair's q/k emitted after; the Tile scheduler
                        # pulls these PE groups into attention's bubbles
                        if hp + 1 < NH // 2:
                            for oc in (hp + 1, CT + hp + 1):
                                for tch in range(NCHUNK):
                                    qk_group_fill(oc, tch)
                        rt = npool.tile([P, T], F32R, name=f"resid{hp}", tag=f"resid{hp}")
                        nc.vector.tensor_add(out=rt, in0=x_sb[hp], in1=a_sb[hp])
                        r_sb[hp] = rt
                else:
                    for hp in range(NH // 2):
                        if hp + 1 < NH // 2:
                            for oc in (hp + 1, CT + hp + 1):
                                for tch in range(NCHUNK):
                                    qk_group_fill(oc, tch)
                    for i in range(CT):
                        ot = qkp.tile([P, T], F32, name=f"o{i}", tag=f"q{i}")
                        nc.vector.tensor_copy(out=ot, in_=q_sb[i])
                        nc.sync.dma_start(out=out_d[i * P : (i + 1) * P, :], in_=ot)
                    return

                # ---- projection ----
                for oc in range(CT):
                    ot = qkp.tile([P, T], F32, name=f"o{oc}", tag=f"q{oc}")
                    ps = pswp.tile([P, 1024], F32, name="pso", tag="psw")
                    for ci in range(CT):
                        for tch in range(NCHUNK):
                            nc.tensor.matmul(
                                ps[:, tch * 512 : (tch + 1) * 512],
                                lhsT=pw_sb[ci][:, oc * P : (oc + 1) * P],
                                rhs=r_sb[ci][:, tch * 512 : (tch + 1) * 512],
                                start=(ci == 0), stop=(ci == CT - 1),
                            )
                    if oc % 2 == 0:
                        # ACT is idle post-attention; share the evict chain
                        nc.scalar.activation(
                            out=ot, in_=ps, func=AF.Identity,
                            bias=vec_sb[oc][:, 4:5], scale=1.0,
                        )
                    else:
                        nc.vector.tensor_scalar_add(
                            out=ot, in0=ps, scalar1=vec_sb[oc][:, 4:5],
                        )
                    # split the store across queues so the final drain overlaps
                    oeng = [(nc.sync, nc.scalar), (nc.gpsimd, nc.sync),
                            (nc.scalar, nc.gpsimd), (nc.sync, nc.scalar)][oc]
                    for half in range(2):
                        oeng[half].dma_start(
                            out=out_d[oc * P : (oc + 1) * P, half * 512 : (half + 1) * 512],
                            in_=ot[:, half * 512 : (half + 1) * 512],
                        )


            if loop_n:
                with tc.For_i(0, loop_n, 1, staggered_reset=True):
                    body()
            else:
                body()

    nc.compile()
    return nc


def _prep_inputs(x, gn_w, gn_b, qkv_w, qkv_b, proj_w, proj_b):
    scale = 1.0 / np.sqrt(CH)  # both 1/ch^0.25 factors folded into q
    wq = qkv_w[0:C] * scale
    wk = qkv_w[C : 2 * C]
    wv = qkv_w[2 * C : 3 * C]
    bq = qkv_b[0:C] * scale
    bk = qkv_b[C : 2 * C]
    bv = qkv_b[2 * C : 3 * C]
    wqkvT = round_fp32r(np.concatenate([wq, wk, wv], axis=0).T)  # [C, 3C]
    pwT_a = round_fp32r(proj_w.T)  # [C, C]
    pb2 = proj_b + proj_w.astype(np.float64) @ bv.astype(np.float64)
    vecs = np.stack(
        [bq, bk, gn_w, gn_b, pb2.astype(np.float32)], axis=-1
    ).reshape(CT, P, 5).astype(np.float32)
    maskD = np.zeros((C, NG), dtype=np.float32)
    for c in range(C):
        maskD[c, c // GS] = 1.0 / GS
    maskU = np.zeros((NG, C), dtype=np.float32)
    for c in range(C):
        maskU[c // GS, c] = 1.0
    shared = {
        "wqkvT": np.ascontiguousarray(wqkvT),
        "pwT": np.ascontiguousarray(pwT_a),
        "vecs": np.ascontiguousarray(vecs),
        "maskD": maskD,
        "maskU": maskU,
    }
    in_maps = []
    for b in range(B):
        m = dict(shared)
        m["xin"] = np.ascontiguousarray(x[b].reshape(C, T).astype(np.float32))
        in_maps.append(m)
    return in_maps


def run(inputs, trace=False):
    from concourse import bass_utils

    if "nc" not in _CACHE:
        _CACHE["nc"] = _build()
    nc = _CACHE["nc"]
    in_maps = _prep_inputs(**{k: np.asarray(v) for k, v in inputs.items()})
    res = bass_utils.run_bass_kernel_spmd(
        nc, in_maps, core_ids=list(range(B)), trace=trace
    )
    out = np.stack([res.results[b]["out"].reshape(C, H, W) for b in range(B)])
    return out, res


def kernel(**inputs) -> np.ndarray:
    out, _ = run(inputs, trace=False)
    return out

